# revision 7
# baseline (speedup 1.0000x reference)
"""Trainium2 Bass kernel for nn_GaussianSelfAttention (B=64, S=197, D=768).

Math: the reference's softmax is over a singleton axis, so attn == 1.0 exactly
and out = concat([ones(B,1,D), sample_v], axis=1) where
sample_v = (G @ x) @ Wv + wsum*bv,  G = per-image (196,197) bilinear one-hot
matrix built from Gaussian-sampled keys. q/k projections are dead code.

Device strategy (8 cores, data-parallel over batch, 8 images/core):
  - single big DMA per tensor; host pre-packs tiles into partition-major form
  - key/weight/index math on DVE in fp32 (exact floor via int-roundtrip)
  - one-hot rows built with fused tensor_scalar(is_equal, mult) on DVE
  - pairwise point-set combine on GPSIMD, then accumulating PE transposes -> GT
  - sxT = x.T-gather via matmul(lhsT=x, rhs=GT) in f32r (N padded to 256)
  - sv = sxT.T @ Wv in f32r, staged in groups, scrambled layout unpacked on host
"""

import numpy as np

import concourse.bass as bass
import concourse.mybir as mybir
import concourse.tile as tile
from concourse import bacc, bass_utils
from concourse.masks import make_identity

B, S, D, P = 64, 197, 768, 196
N_CORES = 8
BPC = B // N_CORES            # images per core
ROWS = BPC * S                # 1576 input/output rows per core
Q = BPC * P                   # 1568 sampled rows per core
GRID = 14.0
NF = 198                      # padded one-hot free size (even, >= S)
NPAD = 256                    # padded gather-matmul N (f32r full rate)
NM = (Q + 127) // 128         # 13 output m-chunks
OGRP = [(0, 4), (4, 4), (8, 4), (12, 1)]  # output DMA groups (start, len)

F32 = mybir.dt.float32
F32R = mybir.dt.float32r
I32 = mybir.dt.int32
OP = mybir.AluOpType

PCH = [(0, 128), (128, 68)]   # partition chunks of P=196
SCH = [(0, 128), (128, 69)]   # chunks of S=197

_NC = {}
_RUNNER = {}


def _emit(nc, iters=1):
    x_d = nc.dram_tensor("x0", (128, 2 * BPC * D), F32R, kind="ExternalInput")
    wv_d = nc.dram_tensor("wv0", (128, 6 * D), F32R, kind="ExternalInput")
    pr_d = nc.dram_tensor("pr0", (128, 96), F32, kind="ExternalInput")
    o_d = nc.dram_tensor("o0", (128, NM * D), F32, kind="ExternalOutput")

    with tile.TileContext(nc) as tc:
        with (
            tc.tile_pool(name="const", bufs=1) as cpool,
            tc.tile_pool(name="xb", bufs=1) as xpool,
            tc.tile_pool(name="wvp", bufs=1) as wpool,
            tc.tile_pool(name="sxp", bufs=1) as spool,
            tc.tile_pool(name="km", bufs=1) as kpool,
            tc.tile_pool(name="gp", bufs=2) as gpool,
            tc.tile_pool(name="gtp", bufs=2) as gtpool,
            tc.tile_pool(name="ost", bufs=2) as opool,
            tc.tile_pool(name="psT", bufs=2, space="PSUM") as psT,
            tc.tile_pool(name="psA", bufs=3, space="PSUM") as psA,
            tc.tile_pool(name="psB", bufs=3, space="PSUM") as psB,
        ):
            # ---- constants (outside the timing loop) ----
            ident = cpool.tile([128, 128], F32, name="ident", tag="ident")
            make_identity(nc, ident[:])
            iotaf = cpool.tile([128, NF], F32, name="iota", tag="iota")
            nc.gpsimd.iota(iotaf[:], pattern=[[1, NF]], base=0,
                           channel_multiplier=0,
                           allow_small_or_imprecise_dtypes=True)
            zpad = cpool.tile([128, NPAD - P], F32, name="zpad", tag="zpad")
            nc.vector.memset(zpad[:], 0.0)

            def body():
                # ---- single-DMA loads ----
                xall = xpool.tile([128, 2 * BPC * D], F32R, name="xall",
                                  tag="xall")
                nc.sync.dma_start(out=xall[:], in_=x_d[:])
                wvt = wpool.tile([128, 6 * D], F32R, name="wvt", tag="wvt")
                nc.sync.dma_start(out=wvt[:], in_=wv_d[:])
                prt = kpool.tile([128, 96], F32, name="prt", tag="prt")
                nc.sync.dma_start(out=prt[:], in_=pr_d[:])

                def xsl(b, c, mj):   # lhsT slice of x image b, s-chunk c
                    t = 2 * b + c
                    pn = 128 if c == 0 else 69
                    return xall[0:pn, t * D + mj * 128: t * D + (mj + 1) * 128]

                sxT = [spool.tile([128, Q], F32R, name=f"sxT{kc}",
                                  tag=f"sxT{kc}") for kc in range(6)]

                # ---- key / weight / index math ----
                w4 = []
                i4 = []
                for c, (p0, pn) in enumerate(PCH):
                    def col(j):
                        return prt[0:pn, c * 48 + j * BPC: c * 48 + (j + 1) * BPC]

                    def tl(tag, dt=F32):
                        return kpool.tile([pn, BPC], dt, name=f"{tag}{c}",
                                          tag=f"{tag}{c}")

                    keys = []
                    for (jn, ja, js) in ((0, 2, 4), (1, 3, 5)):
                        k = tl(f"k{jn}")
                        nc.vector.tensor_tensor(out=k[:], in0=col(jn),
                                                in1=col(ja), op=OP.subtract)
                        rs = tl(f"rs{jn}")
                        nc.vector.reciprocal(rs[:], col(js))
                        nc.vector.tensor_tensor(out=k[:], in0=k[:], in1=rs[:],
                                                op=OP.mult)
                        keys.append(k)
                    kx, ky = keys

                    cells = {}
                    for nm, k in (("x", kx), ("y", ky)):
                        ti = tl(f"ti{nm}", I32)
                        nc.vector.tensor_copy(out=ti[:], in_=k[:])
                        tf = tl(f"tf{nm}")
                        nc.vector.tensor_copy(out=tf[:], in_=ti[:])
                        corr = tl(f"co{nm}")
                        nc.vector.tensor_tensor(out=corr[:], in0=tf[:],
                                                in1=k[:], op=OP.is_gt)
                        flo = tl(f"fl{nm}")
                        nc.vector.tensor_tensor(out=flo[:], in0=tf[:],
                                                in1=corr[:], op=OP.subtract)
                        up = tl(f"up{nm}")
                        nc.vector.tensor_tensor(out=up[:], in0=k[:], in1=flo[:],
                                                op=OP.is_gt)
                        cei = tl(f"ce{nm}")
                        nc.vector.tensor_tensor(out=cei[:], in0=flo[:],
                                                in1=up[:], op=OP.add)
                        dc = tl(f"dc{nm}")
                        nc.vector.tensor_tensor(out=dc[:], in0=cei[:], in1=k[:],
                                                op=OP.subtract)
                        wc = tl(f"wc{nm}")
                        nc.vector.tensor_scalar(out=wc[:], in0=dc[:],
                                                scalar1=-1.0, scalar2=1.0,
                                                op0=OP.mult, op1=OP.add)
                        df = tl(f"df{nm}")
                        nc.vector.tensor_tensor(out=df[:], in0=k[:], in1=flo[:],
                                                op=OP.subtract)
                        wf = tl(f"wf{nm}")
                        nc.vector.tensor_scalar(out=wf[:], in0=df[:],
                                                scalar1=-1.0, scalar2=1.0,
                                                op0=OP.mult, op1=OP.add)
                        cells[nm] = (cei, flo, wc, wf)

                    x1, x2, wx1, wx2 = cells["x"]
                    y1, y2, wy1, wy2 = cells["y"]

                    w4c = kpool.tile([pn, 4 * BPC], F32, name=f"w4{c}",
                                     tag=f"w4{c}")
                    i4c = kpool.tile([pn, 4 * BPC], F32, name=f"i4{c}",
                                     tag=f"i4{c}")
                    fy = {}
                    for nm, yy in (("y1", y1), ("y2", y2)):
                        f = tl(f"fy{nm}")
                        nc.vector.tensor_scalar(out=f[:], in0=yy[:],
                                                scalar1=GRID, scalar2=None,
                                                op0=OP.mult)
                        fy[nm] = f
                    combos = [(x1, wx1, "y1", wy1), (x2, wx2, "y1", wy1),
                              (x1, wx1, "y2", wy2), (x2, wx2, "y2", wy2)]
                    for ci, (xx, wxx, ynm, wyy) in enumerate(combos):
                        sl = slice(ci * BPC, (ci + 1) * BPC)
                        nc.vector.tensor_tensor(out=w4c[:, sl], in0=wxx[:],
                                                in1=wyy[:], op=OP.mult)
                        f = tl(f"f{ci}")
                        nc.vector.tensor_tensor(out=f[:], in0=fy[ynm][:],
                                                in1=xx[:], op=OP.add)
                        wr = tl(f"wr{ci}")
                        nc.vector.tensor_scalar(out=wr[:], in0=f[:],
                                                scalar1=0.0, scalar2=float(S),
                                                op0=OP.is_lt, op1=OP.mult)
                        nc.vector.tensor_tensor(out=i4c[:, sl], in0=f[:],
                                                in1=wr[:], op=OP.add)
                    w4.append(w4c)
                    i4.append(i4c)

                # ---- per image: one-hots -> GPSIMD combine -> PE transposes
                for b in range(BPC):
                    gcs = []
                    for c, (p0, pn) in enumerate(PCH):
                        g4 = []
                        for ci in range(4):
                            g = gpool.tile([pn, NF], F32, name=f"g{c}_{ci}",
                                           tag=f"g{c}_{ci}")
                            nc.vector.tensor_scalar(
                                out=g[:], in0=iotaf[:pn, :],
                                scalar1=i4[c][:, ci * BPC + b:ci * BPC + b + 1],
                                scalar2=w4[c][:, ci * BPC + b:ci * BPC + b + 1],
                                op0=OP.is_equal, op1=OP.mult)
                            g4.append(g)
                        ga = gpool.tile([pn, NF], F32, name=f"ga{c}",
                                        tag=f"ga{c}")
                        nc.gpsimd.tensor_tensor(out=ga[:], in0=g4[0][:],
                                                in1=g4[1][:], op=OP.add)
                        gb = gpool.tile([pn, NF], F32, name=f"gb{c}",
                                        tag=f"gb{c}")
                        nc.gpsimd.tensor_tensor(out=gb[:], in0=g4[2][:],
                                                in1=g4[3][:], op=OP.add)
                        gcs.append((ga, gb))

                    gt0 = gtpool.tile([128, NPAD], F32R, name="gt0", tag="gt0")
                    gt1 = gtpool.tile([69, NPAD], F32R, name="gt1", tag="gt1")
                    nc.vector.tensor_copy(out=gt0[:, P:NPAD], in_=zpad[:, :])
                    nc.vector.tensor_copy(out=gt1[:, P:NPAD], in_=zpad[:69, :])
                    for sc, (s0, sn) in enumerate(SCH):
                        gt = (gt0, gt1)[sc]
                        for c, (p0, pn) in enumerate(PCH):
                            pt = psT.tile([sn, pn], F32, name="pt", tag="pt")
                            for gi, gg in enumerate(gcs[c]):
                                nc.tensor.matmul(pt[:], lhsT=gg[:, s0:s0 + sn],
                                                 rhs=ident[:pn, :pn],
                                                 is_transpose=True,
                                                 start=(gi == 0), stop=(gi == 1))
                            nc.scalar.copy(out=gt[:, p0:p0 + pn], in_=pt[:])

                    # ---- gather matmul (f32r, N=256) ----
                    for mj in range(6):
                        pa = psA.tile([128, NPAD], F32, name="pa", tag="pa")
                        nc.tensor.matmul(pa[:], lhsT=xsl(b, 0, mj), rhs=gt0[:],
                                         start=True, stop=False)
                        nc.tensor.matmul(pa[:], lhsT=xsl(b, 1, mj), rhs=gt1[:],
                                         start=False, stop=True)
                        eng = nc.vector.tensor_copy if mj % 2 else nc.scalar.copy
                        eng(out=sxT[mj][:, b * P:(b + 1) * P], in_=pa[:, 0:P])

                # ---- projection matmul + grouped output DMA ----
                for g0, glen in OGRP:
                    gw = glen * D
                    st = opool.tile([128, gw], F32, name="ost", tag="ost")
                    mp_last = 128
                    for j in range(glen):
                        mi = g0 + j
                        q0 = mi * 128
                        mp = min(128, Q - q0)
                        mp_last = mp
                        for n in range(2):
                            pb = psB.tile([128, 384], F32, name="pb", tag="pb")
                            for kc in range(6):
                                nc.tensor.matmul(
                                    pb[:mp, :], lhsT=sxT[kc][:, q0:q0 + mp],
                                    rhs=wvt[:, kc * D + n * 384:
                                            kc * D + (n + 1) * 384],
                                    start=(kc == 0), stop=(kc == 5))
                            eng = nc.vector.tensor_copy if n == 0 else nc.scalar.copy
                            eng(out=st[:mp, j * D + n * 384:j * D + (n + 1) * 384],
                                in_=pb[:mp, :])
                    rows = 128 if glen > 1 else mp_last
                    nc.sync.dma_start(out=o_d[0:rows, g0 * D:g0 * D + gw],
                                      in_=st[0:rows, :])

            if iters == 1:
                body()
            else:
                with tc.For_i(0, iters, 1):
                    body()


def _build(iters=1):
    if iters not in _NC:
        nc = bacc.Bacc("TRN2", target_bir_lowering=False, debug=False,
                       num_devices=N_CORES)
        _emit(nc, iters)
        nc.compile()
        _NC[iters] = nc
    return _NC[iters]


def _pack_inputs(x, img_ids, Wv, avgs, std_devs, noise):
    x = np.asarray(x, np.float32)
    wv = np.asarray(Wv, np.float32)
    wvp = np.ascontiguousarray(
        wv.reshape(6, 128, D).transpose(1, 0, 2).reshape(128, 6 * D))
    ids = np.asarray(img_ids).astype(np.int64)
    avgs = np.asarray(avgs, np.float32)
    std_devs = np.asarray(std_devs, np.float32)
    noise = np.asarray(noise, np.float32)
    in_maps = []
    for c in range(N_CORES):
        sl = slice(c * BPC, (c + 1) * BPC)
        xs = x[sl].reshape(ROWS, D)
        xt = np.zeros((2 * BPC, 128, D), np.float32)
        for b in range(BPC):
            xt[2 * b] = xs[b * S:b * S + 128]
            xt[2 * b + 1, :69] = xs[b * S + 128:(b + 1) * S]
        xp = np.ascontiguousarray(
            xt.transpose(1, 0, 2).reshape(128, 2 * BPC * D))
        a = avgs[ids[sl]]
        s = std_devs[ids[sl]]
        nz = noise[sl]
        prm = np.empty((P, 6, BPC), np.float32)
        prm[:, 0] = nz[:, 0].T
        prm[:, 1] = nz[:, 1].T
        prm[:, 2] = a[:, 0].T
        prm[:, 3] = a[:, 1].T
        prm[:, 4] = s[:, 0].T
        prm[:, 5] = s[:, 1].T
        prf = prm.reshape(P, 48)
        pp = np.ones((128, 96), np.float32)
        pp[:, 0:48] = prf[0:128]
        pp[0:68, 48:96] = prf[128:196]
        in_maps.append({"x0": xp, "wv0": wvp, "pr0": pp})
    return in_maps


def _unpack_out(o_np):
    # o_np: (128, NM*D) scrambled m-chunk layout -> (BPC, S, D) with ones rows
    sv = o_np.reshape(128, NM, D).transpose(1, 0, 2).reshape(NM * 128, D)[:Q]
    out = np.ones((BPC, S, D), np.float32)
    out[:, 1:, :] = sv.reshape(BPC, P, D)
    return out


def _get_runner(iters=1):
    """Build the sharded PJRT callable once and cache it."""
    if iters in _RUNNER:
        return _RUNNER[iters]
    import jax
    from jax.experimental.shard_map import shard_map
    from jax.sharding import Mesh, PartitionSpec
    from concourse import bass2jax, mybir as _mybir

    nc = _build(iters)
    bass2jax.install_neuronx_cc_hook()
    in_names, out_names, out_avals, zero_outs = [], [], [], []
    part_name = (nc.partition_id_tensor.name
                 if nc.partition_id_tensor else None)
    for alloc in nc.m.functions[0].allocations:
        if not isinstance(alloc, _mybir.MemoryLocationSet):
            continue
        name = alloc.memorylocations[0].name
        if alloc.kind == "ExternalInput":
            if name != part_name:
                in_names.append(name)
        elif alloc.kind == "ExternalOutput":
            shape = tuple(alloc.tensor_shape)
            dtype = _mybir.dt.np(alloc.dtype)
            out_names.append(name)
            out_avals.append(jax.core.ShapedArray(shape, dtype))
            zero_outs.append(np.zeros(shape, dtype))
    n_params = len(in_names)
    all_names = in_names + out_names
    if part_name is not None:
        all_names = all_names + [part_name]
    donate = tuple(range(n_params, n_params + len(out_names)))

    def _body(*args):
        operands = list(args)
        if part_name is not None:
            operands.append(bass2jax.partition_id_tensor())
        outs = bass2jax._bass_exec_p.bind(
            *operands,
            out_avals=tuple(out_avals),
            in_names=tuple(all_names),
            out_names=tuple(out_names),
            lowering_input_output_aliases=(),
            sim_require_finite=True,
            sim_require_nnan=True,
            nc=nc,
        )
        return tuple(outs)

    devices = jax.devices()[:N_CORES]
    mesh = Mesh(np.asarray(devices), ("core",))
    specs = (PartitionSpec("core"),) * (n_params + len(out_names))
    fn = jax.jit(
        shard_map(_body, mesh=mesh, in_specs=specs,
                  out_specs=(PartitionSpec("core"),) * len(out_names),
                  check_rep=False),
        donate_argnums=donate, keep_unused=True)

    def run(in_maps):
        concat_in = [
            np.concatenate([np.asarray(m[nm]) for m in in_maps], axis=0)
            for nm in in_names
        ]
        concat_zero = [
            np.zeros((N_CORES * z.shape[0], *z.shape[1:]), z.dtype)
            for z in zero_outs
        ]
        arrs = fn(*concat_in, *concat_zero)
        return [
            {nm: np.asarray(arrs[i]).reshape(N_CORES, *out_avals[i].shape)[c]
             for i, nm in enumerate(out_names)}
            for c in range(N_CORES)
        ]

    _RUNNER[iters] = run
    return run


class _Res:
    def __init__(self, results):
        self.results = results
        self.exec_time_ns = None


def run_cores(in_maps, trace=False, iters=1):
    return _Res(_get_runner(iters)(in_maps))


def kernel(x, img_ids, mask=None, Wq=None, bq=None, Wk=None, bk=None,
           Wv=None, bv=None, avgs=None, std_devs=None, noise=None,
           _trace=False, _results=None):
    in_maps = _pack_inputs(x, img_ids, Wv, avgs, std_devs, noise)
    res = run_cores(in_maps, trace=_trace)
    if _results is not None:
        _results.append(res)
    out = np.concatenate(
        [_unpack_out(res.results[c]["o0"]) for c in range(N_CORES)], axis=0)
    bv_np = np.asarray(bv, np.float32) if bv is not None else None
    if bv_np is not None and np.any(bv_np):
        # sample() is affine: add (sum_i w_i) * bv for the sampled rows.
        ids = np.asarray(img_ids).astype(np.int64)
        a = np.asarray(avgs, np.float32)[ids]
        sd = np.asarray(std_devs, np.float32)[ids]
        nz = np.asarray(noise, np.float32)
        kx = (nz[:, 0] - a[:, 0]) / sd[:, 0]
        ky = (nz[:, 1] - a[:, 1]) / sd[:, 1]
        fx1, fx2 = np.ceil(kx), np.floor(kx)
        fy1, fy2 = np.ceil(ky), np.floor(ky)
        wsum = ((1 - np.abs(fx1 - kx)) * (1 - np.abs(fy1 - ky))
                + (1 - np.abs(fx2 - kx)) * (1 - np.abs(fy1 - ky))
                + (1 - np.abs(fx1 - kx)) * (1 - np.abs(fy2 - ky))
                + (1 - np.abs(fx2 - kx)) * (1 - np.abs(fy2 - ky)))
        out[:, 1:, :] += wsum[:, :, None] * bv_np[None, None, :]
    return out
